# revision 24
# baseline (speedup 1.0000x reference)
"""Graph Wavelet NN (2-layer) Trainium2 kernel, 8-core row-parallel.

Math per layer: out = (wavelets * f) @ (wavelets_inv @ (x @ W)); the filter is
folded into a row-scale of the small spectral tensor s computed per core.

Final design (measured ~444-460us vs the 477us v1 baseline on the same
trace-based metric; run-to-run variance tracks the ~40-55us cross-core launch
skew absorbed by the first collective):
- t1 = x @ W1 computed FULLY REPLICATED per core, with s1's first TWO
  column-quarter passes interleaved into the t1 granule loop (lagged 3/9
  granules behind the xq stream).  The PE never idles long enough to
  re-throttle the HAM clock gate, layer 1 needs no input exchange, and
  E2-h0 (the skew-absorbing first AllGather) fires as early as possible.
- winvT is SBUF-RESIDENT (16MB, filled during t1 from a host-prepared
  col-quarter-major layout in 8 contiguous 2MB DMAs, reused by s2); wT is
  streamed during o1 and o2, with the first 4MB of each o-phase pre-staged
  into t1f's dead SBUF slot (wt_head1/2, head2 trickled into o1's stream).
- ONE total order sigma = (half, rank) over the 64 contraction blocks is
  baked into the host layouts of xT, winvT, wT; every DMA is a contiguous
  block read and every consumer walks gathered rank-halves (256KB reads,
  own rank included) in AllGather arrival order -> fully static program,
  no dynamic-offset DMAs (their register setup paced earlier versions).
- All exchanges are HALVES (256KB in / 2MB out per AllGather): s1 AGs each
  half as its passes finish; t2 AGs per half; s2 runs as TWO output-half
  passes so E4-h0 flies mid-s2 and o2 starts stall-free.  o1/o2 are
  slot-pipelined single passes.  A dummy AllGather at t=0 absorbs the
  one-time ncfw stream init.
- Collectives alone on gpsimd; winv/wT streams on sync; xq/gather reads and
  stores on scalar (s2 granule reads alternate scalar/sync).  bf16 matmuls,
  fp32 PSUM; PSUM: 4x2KB "psA" accumulators + 2x4KB o-phase tiles = 16KB.

Verified on HW: rel err 6.345e-3.
"""

import sys

if "/opt/trn_rl_repo" not in sys.path:
    sys.path.insert(0, "/opt/trn_rl_repo")

import numpy as np
import ml_dtypes

import concourse.bass as bass
import concourse.mybir as mybir
import concourse.tile as tile
from concourse import bacc, bass_utils

N = 8192
F = 512
C = 256
NCORES = 8
R = N // NCORES          # 1024 rows per core
H = R // 2               # 512-row half (exchange granule)
Q = R // 4               # 256-col quarter (s-phase output passes)
NSLOT = N // 128         # 64 contraction slots of 128 rows

F32 = mybir.dt.float32
BF16 = mybir.dt.bfloat16
NP_BF16 = ml_dtypes.bfloat16


def build_kernel(sim_single_core=False):
    nc = bacc.Bacc(
        "TRN2",
        target_bir_lowering=False,
        debug=False,
        num_devices=1 if sim_single_core else NCORES,
    )

    xT = nc.dram_tensor("xT", [16 * F, H], BF16, kind="ExternalInput")
    w1 = nc.dram_tensor("w1", [F, C], BF16, kind="ExternalInput")
    w2 = nc.dram_tensor("w2", [C, C], BF16, kind="ExternalInput")
    winvT = nc.dram_tensor("winvT", [4 * N, Q], BF16, kind="ExternalInput")
    wT = nc.dram_tensor("wT", [N, R], BF16, kind="ExternalInput")
    f1 = nc.dram_tensor("f1", [R], F32, kind="ExternalInput")
    f2 = nc.dram_tensor("f2", [R], F32, kind="ExternalInput")
    outT = nc.dram_tensor("outT", [C, R], F32, kind="ExternalOutput")

    rg = [list(range(NCORES))]

    with tile.TileContext(nc) as tc:
        with (
            tc.tile_pool(name="dram", bufs=1, space="DRAM") as dram,
            tc.tile_pool(name="const", bufs=1) as const,
            tc.tile_pool(name="stgp", bufs=2) as stgp,
            tc.tile_pool(name="t1fp", bufs=1) as t1fp,
            tc.tile_pool(name="wtp", bufs=3) as wtp,
            tc.tile_pool(name="tsp", bufs=3) as tsp,
            tc.tile_pool(name="psq", bufs=4, space="PSUM") as psq,
            tc.tile_pool(name="psO", bufs=2, space="PSUM") as psO,
        ):
            # ---- DRAM exchange buffers (halves) ----
            def mk_pair(nm):
                ins, outs = [], []
                for b in range(2):
                    ins.append(dram.tile([H, C], BF16, name=f"{nm}{b}_d"))
                    outs.append(
                        dram.tile(
                            [NCORES * H, C], BF16,
                            addr_space="Local" if sim_single_core else "Shared",
                            name=f"{nm}{b}g_d",
                        )
                    )
                return ins, outs

            s1h_d, s1g_d = mk_pair("s1")
            t2h_d, t2g_d = mk_pair("t2")
            s2h_d, s2g_d = mk_pair("s2")

            # dummy collective: starts the ncfw stream init at t~0 so the
            # first real AllGather doesn't eat the one-time cost.
            dum_i = dram.tile([Q, 1], BF16, name="dum_i")
            dum_o = dram.tile(
                [NCORES * Q, 1], BF16,
                addr_space="Local" if sim_single_core else "Shared",
                name="dum_o",
            )

            # ---- persistent SBUF ----
            winv_sb = const.tile([128, 4, NSLOT, Q], BF16)  # 128KB/part
            t1f_sb = t1fp.tile(
                [128, NSLOT, C], BF16, tag="big", name="t1f_sb"
            )  # full t1, sigma order
            w1_sb = const.tile([128, F // 128, C], BF16)
            w2_sb = const.tile([128, C // 128, C], BF16)
            f1_sb = const.tile([128, 8], F32)
            f2_sb = const.tile([128, 8], F32)

            def all_gather(in_d, out_d):
                if sim_single_core:
                    rows = in_d.shape[0]
                    for rr in range(NCORES):
                        nc.sync.dma_start(
                            out=out_d[rr * rows:(rr + 1) * rows, :], in_=in_d[:, :]
                        )
                else:
                    nc.gpsimd.collective_compute(
                        "AllGather",
                        mybir.AluOpType.bypass,
                        replica_groups=rg,
                        ins=[in_d.opt()],
                        outs=[out_d.opt()],
                    )

            all_gather(dum_i, dum_o)

            nc.scalar.dma_start(
                out=w1_sb[:], in_=w1.ap().rearrange("(kc p) n -> p kc n", p=128)
            )

            # winv fill: col-quarter-major 2MB pieces, each contiguous.
            for q in range(4):
                for g2 in range(2):
                    nc.sync.dma_start(
                        out=winv_sb[:, q, g2 * 32:(g2 + 1) * 32, :],
                        in_=winvT.ap()[
                            q * N + g2 * 4096:q * N + (g2 + 1) * 4096, :
                        ].rearrange("(kc p) m -> p kc m", p=128),
                    )

            # ======= t1 = x @ W1, fully replicated, staged in sigma order =====
            # xq granule gi = 512 sigma-columns (4 slots), contiguous 512KB.
            # s1's pass-0 matmuls are interleaved 3 granules behind t1 so the
            # PE never idles between xq arrivals (idle gaps re-throttle the
            # HAM clock gate and were running t1 at half rate).
            ps01 = [
                psq.tile([128, 2, C], F32, tag="psA", name=f"ps1_{qq}")
                for qq in range(2)
            ]

            def s1q_mm(qq, c):
                for j in range(2):
                    nc.tensor.matmul(
                        ps01[qq][:, j, :],
                        winv_sb[:, qq, c, j * 128:(j + 1) * 128],
                        t1f_sb[:, c, :],
                        start=(c == 0 and j == 0),
                        stop=(c == NSLOT - 1),
                        skip_group_check=True,
                    )

            for gi in range(16):
                xq = tsp.tile([128, 4, H], BF16, tag="ts", name=f"xq{gi}")
                nc.scalar.dma_start(
                    out=xq[:],
                    in_=xT.ap()[gi * F:(gi + 1) * F, :].rearrange(
                        "(kc p) m -> p kc m", p=128
                    ),
                )
                for hf in range(2):
                    pt = psq.tile(
                        [128, 2, C], F32, tag="psA", name=f"pt1_{gi}_{hf}"
                    )
                    for j in range(2):
                        jj = 2 * hf + j
                        for kc in range(4):
                            nc.tensor.matmul(
                                pt[:, j, :],
                                xq[:, kc, jj * 128:(jj + 1) * 128],
                                w1_sb[:, kc, :],
                                start=(j == 0 and kc == 0),
                                stop=(kc == 3),
                                skip_group_check=True,
                            )
                        nc.vector.tensor_copy(
                            t1f_sb[:, 4 * gi + jj, :], pt[:, j, :]
                        )
                if gi >= 3:
                    for c in range(4 * (gi - 3), 4 * (gi - 2)):
                        s1q_mm(0, c)
                if gi >= 9:
                    for c in range(8 * (gi - 9), 8 * (gi - 8)):
                        s1q_mm(1, c)
            for c in range(4 * 13, NSLOT):
                s1q_mm(0, c)
            for c in range(8 * 7, NSLOT):
                s1q_mm(1, c)

            nc.scalar.dma_start(
                out=w2_sb[:], in_=w2.ap().rearrange("(kc p) n -> p kc n", p=128)
            )
            nc.scalar.dma_start(
                out=f1_sb[:], in_=f1.ap().rearrange("(mt p) -> p mt", p=128)
            )
            nc.scalar.dma_start(
                out=f2_sb[:], in_=f2.ap().rearrange("(mt p) -> p mt", p=128)
            )


            # ======= s1 = Winv @ t1 (all SBUF), col-quarter passes; AG halves =
            s_sb1 = stgp.tile([128, 8, C], BF16, tag="stg", name="s_sb1")
            for qq in range(2):
                for j in range(2):
                    nc.vector.tensor_scalar_mul(
                        s_sb1[:, 2 * qq + j, :],
                        ps01[qq][:, j, :],
                        f1_sb[:, 2 * qq + j:2 * qq + j + 1],
                    )
            h = 0
            nc.scalar.dma_start(
                out=s1h_d[h][:, :].rearrange("(k p) n -> p k n", p=128),
                in_=s_sb1[:, 0:4, :],
            )
            all_gather(s1h_d[h], s1g_d[h])
            for q in range(2, 4):
                ps = psq.tile([128, 2, C], F32, tag="psA", name=f"ps1_{q}")
                for p in range(NSLOT):
                    for j in range(2):
                        nc.tensor.matmul(
                            ps[:, j, :],
                            winv_sb[:, q, p, j * 128:(j + 1) * 128],
                            t1f_sb[:, p, :],
                            start=(p == 0 and j == 0),
                            stop=(p == NSLOT - 1),
                            skip_group_check=True,
                        )
                for j in range(2):
                    nc.vector.tensor_scalar_mul(
                        s_sb1[:, 2 * q + j, :],
                        ps[:, j, :],
                        f1_sb[:, 2 * q + j:2 * q + j + 1],
                    )
                if q == 3:
                    nc.scalar.dma_start(
                        out=s1h_d[1][:, :].rearrange("(k p) n -> p k n", p=128),
                        in_=s_sb1[:, 4:8, :],
                    )
                    all_gather(s1h_d[1], s1g_d[1])

            # ---- o phase: out_loc = (w[rows]*f) @ s_full, slot-pipelined ----
            # consumes gathered rank-halves (256KB contiguous) in sigma order.
            def o_phase(sg_d, drain_cb, name, head, side=None):
                po = [
                    psO.tile([128, R], F32, tag="po", name=f"po_{name}{ch}")
                    for ch in range(2)
                ]
                wt_tiles = {}

                def load_wt(g):
                    t = wtp.tile([128, 4, R], BF16, tag="wt", name=f"wt_{name}{g}")
                    nc.sync.dma_start(
                        out=t[:],
                        in_=wT.ap()[g * 512:(g + 1) * 512, :].rearrange(
                            "(kc p) m -> p kc m", p=128
                        ),
                    )
                    wt_tiles[g] = t

                def wt_of(c, mh):
                    # slots 0..15 come from the pre-staged 4MB head tile
                    if c < 16:
                        return head[:, c, mh * 512:(mh + 1) * 512]
                    g, jj = divmod(c, 4)
                    return wt_tiles[g][:, jj, mh * 512:(mh + 1) * 512]

                for h in range(2):
                    for rk in range(NCORES):
                        g = h * 8 + rk
                        sgt = tsp.tile(
                            [128, 4, C], BF16, tag="ts", name=f"so_{name}_{g}"
                        )
                        nc.scalar.dma_start(
                            out=sgt[:],
                            in_=sg_d[h][rk * H:(rk + 1) * H, :].rearrange(
                                "(k p) n -> p k n", p=128
                            ),
                        )
                        if 4 <= g + 2 < 16:
                            load_wt(g + 2)
                        if side is not None and g in (4, 7, 10, 13):
                            side((g - 4) // 3)
                        for jj in range(4):
                            c = 4 * g + jj
                            for ch in range(2):
                                for mh in range(2):
                                    nc.tensor.matmul(
                                        po[ch][:, mh * 512:(mh + 1) * 512],
                                        sgt[:, jj, ch * 128:(ch + 1) * 128],
                                        wt_of(c, mh),
                                        start=(c == 0),
                                        stop=(c == NSLOT - 1),
                                        skip_group_check=True,
                                    )
                for ch in range(2):
                    drain_cb(ch, po[ch])

            # ================= layer 1 out =================
            _h1 = {}

            def relu_drain(ch, po):
                if "t" not in _h1:
                    _h1["t"] = stgp.tile(
                        [128, C // 128, R], BF16, tag="stg", name="h1T_sb"
                    )
                h1T_sb = _h1["t"]
                for mh in range(2):
                    nc.vector.tensor_scalar_max(
                        h1T_sb[:, ch, mh * 512:(mh + 1) * 512],
                        po[:, mh * 512:(mh + 1) * 512],
                        0.0,
                    )

            wt_head1 = t1fp.tile([128, 16, R], BF16, tag="big", name="wt_head1")
            nc.sync.dma_start(
                out=wt_head1[:],
                in_=wT.ap()[0:2048, :].rearrange("(kc p) m -> p kc m", p=128),
            )
            wt_head2 = t1fp.tile([128, 16, R], BF16, tag="big", name="wt_head2")

            def o1_side(i):
                nc.sync.dma_start(
                    out=wt_head2[:, 4 * i:4 * (i + 1), :],
                    in_=wT.ap()[i * 512:(i + 1) * 512, :].rearrange(
                        "(kc p) m -> p kc m", p=128
                    ),
                )

            o_phase(s1g_d, relu_drain, "o1", wt_head1, side=o1_side)
            h1T_sb = _h1["t"]

            # ======= t2 = relu(o1) @ W2 (local rows), AG per half =======
            t_sb2 = stgp.tile([128, 8, C], BF16, tag="stg", name="t_sb2")
            for h in range(2):
                for q2 in range(2):
                    q = 2 * h + q2
                    pt = psq.tile([128, 2, C], F32, tag="psA", name=f"pt2_{q}")
                    for j in range(2):
                        mt = 2 * q + j
                        for kc in range(2):
                            nc.tensor.matmul(
                                pt[:, j, :],
                                h1T_sb[:, kc, mt * 128:(mt + 1) * 128],
                                w2_sb[:, kc, :],
                                start=(j == 0 and kc == 0),
                                stop=(kc == 1),
                                skip_group_check=True,
                            )
                        nc.vector.tensor_copy(t_sb2[:, mt, :], pt[:, j, :])
                nc.scalar.dma_start(
                    out=t2h_d[h][:, :].rearrange("(k p) n -> p k n", p=128),
                    in_=t_sb2[:, 4 * h:4 * h + 4, :],
                )
                all_gather(t2h_d[h], t2g_d[h])

            # ======= s2 = Winv @ t2_full, two output-half passes =======
            # each pass sweeps all 64 slots for rows hp*512..; its half is
            # stored + AllGather'd while the other pass computes, so o2
            # never waits on E4.  Granule re-reads are cheap (winv resident).
            s_sb2 = stgp.tile([128, 8, C], BF16, tag="stg", name="s_sb2")
            for hp in range(2):
                psh = [
                    psq.tile([128, 2, C], F32, tag="psA", name=f"ps2_{hp}_{i}")
                    for i in range(2)
                ]
                for h in range(2):
                    for rk in range(NCORES):
                        g = h * 8 + rk
                        tsg = tsp.tile(
                            [128, 4, C], BF16, tag="ts", name=f"ts2_{hp}_{g}"
                        )
                        (nc.scalar if g % 2 else nc.sync).dma_start(
                            out=tsg[:],
                            in_=t2g_d[h][rk * H:(rk + 1) * H, :].rearrange(
                                "(k p) n -> p k n", p=128
                            ),
                        )
                        for jj in range(4):
                            c = 4 * g + jj
                            for m4 in range(4):
                                mt = 4 * hp + m4
                                nc.tensor.matmul(
                                    psh[m4 // 2][:, m4 % 2, :],
                                    winv_sb[
                                        :, mt // 2, c,
                                        (mt % 2) * 128:(mt % 2 + 1) * 128,
                                    ],
                                    tsg[:, jj, :],
                                    start=(c == 0 and m4 % 2 == 0),
                                    stop=(c == NSLOT - 1),
                                    skip_group_check=True,
                                )
                for m4 in range(4):
                    mt = 4 * hp + m4
                    nc.vector.tensor_scalar_mul(
                        s_sb2[:, mt, :],
                        psh[m4 // 2][:, m4 % 2, :],
                        f2_sb[:, mt:mt + 1],
                    )
                nc.scalar.dma_start(
                    out=s2h_d[hp][:, :].rearrange("(k p) n -> p k n", p=128),
                    in_=s_sb2[:, 4 * hp:4 * hp + 4, :],
                )
                all_gather(s2h_d[hp], s2g_d[hp])

            # ================= layer 2 out =================
            # out_st reuses a "wt" slot; allocated lazily AFTER o2's last wT
            # tile so the ring rotation never makes a wT load wait on the
            # final output stores.
            _oh = {}

            def out_drain(ch, po):
                if "t" not in _oh:
                    _oh["t"] = wtp.tile([128, 2, R], F32, tag="wt", name="out_st")
                out_st = _oh["t"]
                for mh in range(2):
                    nc.vector.tensor_copy(
                        out_st[:, ch, mh * 512:(mh + 1) * 512],
                        po[:, mh * 512:(mh + 1) * 512],
                    )
                    nc.scalar.dma_start(
                        out=outT.ap()[
                            ch * 128:(ch + 1) * 128, mh * 512:(mh + 1) * 512
                        ],
                        in_=out_st[:, ch, mh * 512:(mh + 1) * 512],
                    )

            o_phase(s2g_d, out_drain, "o2", wt_head2)

    nc.compile()
    return nc


_NC_CACHE = {}


def _get_nc():
    if "nc" not in _NC_CACHE:
        _NC_CACHE["nc"] = build_kernel()
    return _NC_CACHE["nc"]


# global sigma order: half-major, rank-major 512-row blocks
_PERM = np.concatenate(
    [
        np.arange(rk * R + h * H, rk * R + h * H + H)
        for h in range(2)
        for rk in range(NCORES)
    ]
)


def make_in_maps(input, wavelets, wavelets_inv, W1, W2, filter1, filter2):
    input = np.asarray(input, np.float32)
    wavelets = np.asarray(wavelets, np.float32)
    wavelets_inv = np.asarray(wavelets_inv, np.float32)
    W1b = np.ascontiguousarray(np.asarray(W1, np.float32)).astype(NP_BF16)
    W2b = np.ascontiguousarray(np.asarray(W2, np.float32)).astype(NP_BF16)
    filter1 = np.asarray(filter1, np.float32)
    filter2 = np.asarray(filter2, np.float32)

    xT_pi = input.T[:, _PERM].astype(NP_BF16)
    # granule-major [16*F, H]: granule gi contiguous; identical on every core
    xT_g = np.ascontiguousarray(
        np.concatenate([xT_pi[:, g * H:(g + 1) * H] for g in range(16)], axis=0)
    )
    in_maps = []
    for i in range(NCORES):
        r0, r1 = i * R, (i + 1) * R
        wvT_pi = wavelets_inv[r0:r1].T[_PERM].astype(NP_BF16)
        winvT_i = np.ascontiguousarray(
            np.concatenate(
                [wvT_pi[:, q * Q:(q + 1) * Q] for q in range(4)], axis=0
            )
        )  # col-quarter-major [4*N, Q]
        wT_i = np.ascontiguousarray(wavelets[r0:r1].T[_PERM]).astype(NP_BF16)
        in_maps.append(
            {
                "xT": xT_g,
                "w1": W1b,
                "w2": W2b,
                "winvT": winvT_i,
                "wT": wT_i,
                "f1": np.ascontiguousarray(filter1[r0:r1]),
                "f2": np.ascontiguousarray(filter2[r0:r1]),
            }
        )
    return in_maps


def run(in_maps, trace=False, **kw):
    nc = _get_nc()
    return bass_utils.run_bass_kernel_spmd(
        nc, in_maps, core_ids=list(range(NCORES)), trace=trace, **kw
    )


def kernel(input, wavelets, wavelets_inv, W1, W2, filter1, filter2):
    in_maps = make_in_maps(
        input, wavelets, wavelets_inv, W1, W2, filter1, filter2
    )
    res = run(in_maps)
    out = np.empty((N, C), np.float32)
    for i in range(NCORES):
        out[i * R:(i + 1) * R, :] = res.results[i]["outT"].T
    return out


# revision 27
# speedup vs baseline: 1.0064x; 1.0064x over previous
"""Graph Wavelet NN (2-layer) Trainium2 kernel, 8-core row-parallel.

Math per layer: out = (wavelets * f) @ (wavelets_inv @ (x @ W)); the filter is
folded into a row-scale of the small spectral tensor s computed per core.

Final design (measured ~444-460us vs the 477us v1 baseline on the same
trace-based metric; run-to-run variance tracks the ~40-55us cross-core launch
skew absorbed by the first collective):
- t1 = x @ W1 computed FULLY REPLICATED per core, with s1's first TWO
  column-quarter passes interleaved into the t1 granule loop (lagged 3/9
  granules behind the xq stream).  The PE never idles long enough to
  re-throttle the HAM clock gate, layer 1 needs no input exchange, and
  E2-h0 (the skew-absorbing first AllGather) fires as early as possible.
- winvT is SBUF-RESIDENT (16MB, filled during t1 from a host-prepared
  col-quarter-major layout in 8 contiguous 2MB DMAs, reused by s2); wT is
  streamed during o1 and o2, with the first 4MB of each o-phase pre-staged
  into t1f's dead SBUF slot (wt_head1/2, head2 trickled into o1's stream).
- ONE total order sigma = (half, rank) over the 64 contraction blocks is
  baked into the host layouts of xT, winvT, wT; every DMA is a contiguous
  block read and every consumer walks gathered rank-halves (256KB reads,
  own rank included) in AllGather arrival order -> fully static program,
  no dynamic-offset DMAs (their register setup paced earlier versions).
- All exchanges are HALVES (256KB in / 2MB out per AllGather): s1 AGs each
  half as its passes finish; t2 AGs per half; s2 runs as TWO output-half
  passes so E4-h0 flies mid-s2 and o2 starts stall-free.  o1/o2 are
  slot-pipelined single passes.  A dummy AllGather at t=0 absorbs the
  one-time ncfw stream init.
- Collectives alone on gpsimd; winv/wT streams on sync; xq/gather reads and
  stores on scalar (s2 granule reads alternate scalar/sync).  bf16 matmuls,
  fp32 PSUM; PSUM: 4x2KB "psA" accumulators + 2x4KB o-phase tiles = 16KB.

Verified on HW: rel err 6.345e-3.
"""

import sys

if "/opt/trn_rl_repo" not in sys.path:
    sys.path.insert(0, "/opt/trn_rl_repo")

import numpy as np
import ml_dtypes

import concourse.bass as bass
import concourse.mybir as mybir
import concourse.tile as tile
from concourse import bacc, bass_utils

N = 8192
F = 512
C = 256
NCORES = 8
R = N // NCORES          # 1024 rows per core
H = R // 2               # 512-row half (exchange granule)
Q = R // 4               # 256-col quarter (s-phase output passes)
NSLOT = N // 128         # 64 contraction slots of 128 rows

F32 = mybir.dt.float32
BF16 = mybir.dt.bfloat16
NP_BF16 = ml_dtypes.bfloat16


def build_kernel(sim_single_core=False):
    nc = bacc.Bacc(
        "TRN2",
        target_bir_lowering=False,
        debug=False,
        num_devices=1 if sim_single_core else NCORES,
    )

    xT = nc.dram_tensor("xT", [16 * F, H], BF16, kind="ExternalInput")
    w1 = nc.dram_tensor("w1", [F, C], BF16, kind="ExternalInput")
    w2 = nc.dram_tensor("w2", [C, C], BF16, kind="ExternalInput")
    winvT = nc.dram_tensor("winvT", [4 * N, Q], BF16, kind="ExternalInput")
    wT = nc.dram_tensor("wT", [N, R], BF16, kind="ExternalInput")
    f1 = nc.dram_tensor("f1", [R], F32, kind="ExternalInput")
    f2 = nc.dram_tensor("f2", [R], F32, kind="ExternalInput")
    outT = nc.dram_tensor("outT", [C, R], F32, kind="ExternalOutput")

    rg = [list(range(NCORES))]

    with tile.TileContext(nc) as tc:
        with (
            tc.tile_pool(name="dram", bufs=1, space="DRAM") as dram,
            tc.tile_pool(name="const", bufs=1) as const,
            tc.tile_pool(name="stgp", bufs=2) as stgp,
            tc.tile_pool(name="t1fp", bufs=1) as t1fp,
            tc.tile_pool(name="wtp", bufs=3) as wtp,
            tc.tile_pool(name="tsp", bufs=3) as tsp,
            tc.tile_pool(name="psq", bufs=4, space="PSUM") as psq,
            tc.tile_pool(name="psO", bufs=2, space="PSUM") as psO,
        ):
            # ---- DRAM exchange buffers (halves) ----
            def mk_pair(nm):
                ins, outs = [], []
                for b in range(2):
                    ins.append(dram.tile([H, C], BF16, name=f"{nm}{b}_d"))
                    outs.append(
                        dram.tile(
                            [NCORES * H, C], BF16,
                            addr_space="Local" if sim_single_core else "Shared",
                            name=f"{nm}{b}g_d",
                        )
                    )
                return ins, outs

            s1h_d, s1g_d = mk_pair("s1")
            t2h_d, t2g_d = mk_pair("t2")
            s2h_d, s2g_d = mk_pair("s2")

            # dummy collective: starts the ncfw stream init at t~0 so the
            # first real AllGather doesn't eat the one-time cost.
            dum_i = dram.tile([Q, 1], BF16, name="dum_i")
            dum_o = dram.tile(
                [NCORES * Q, 1], BF16,
                addr_space="Local" if sim_single_core else "Shared",
                name="dum_o",
            )

            # ---- persistent SBUF ----
            winv_sb = const.tile([128, 4, NSLOT, Q], BF16)  # 128KB/part
            t1f_sb = t1fp.tile(
                [128, NSLOT, C], BF16, tag="big", name="t1f_sb"
            )  # full t1, sigma order
            w1_sb = const.tile([128, F // 128, C], BF16)
            w2_sb = const.tile([128, C // 128, C], BF16)
            f1_sb = const.tile([128, 8], F32)
            f2_sb = const.tile([128, 8], F32)

            def all_gather(in_d, out_d):
                if sim_single_core:
                    rows = in_d.shape[0]
                    for rr in range(NCORES):
                        nc.sync.dma_start(
                            out=out_d[rr * rows:(rr + 1) * rows, :], in_=in_d[:, :]
                        )
                else:
                    nc.gpsimd.collective_compute(
                        "AllGather",
                        mybir.AluOpType.bypass,
                        replica_groups=rg,
                        ins=[in_d.opt()],
                        outs=[out_d.opt()],
                    )

            all_gather(dum_i, dum_o)

            nc.scalar.dma_start(
                out=w1_sb[:], in_=w1.ap().rearrange("(kc p) n -> p kc n", p=128)
            )

            # winv fill: col-quarter-major 2MB pieces, each contiguous.
            for q in range(4):
                for g2 in range(2):
                    nc.sync.dma_start(
                        out=winv_sb[:, q, g2 * 32:(g2 + 1) * 32, :],
                        in_=winvT.ap()[
                            q * N + g2 * 4096:q * N + (g2 + 1) * 4096, :
                        ].rearrange("(p kc) m -> p kc m", p=128),
                    )

            # ======= t1 = x @ W1, fully replicated, staged in sigma order =====
            # xq granule gi = 512 sigma-columns (4 slots), contiguous 512KB.
            # s1's pass-0 matmuls are interleaved 3 granules behind t1 so the
            # PE never idles between xq arrivals (idle gaps re-throttle the
            # HAM clock gate and were running t1 at half rate).
            ps01 = [
                psq.tile([128, 2, C], F32, tag="psA", name=f"ps1_{qq}")
                for qq in range(2)
            ]

            def s1q_mm(qq, c):
                for j in range(2):
                    nc.tensor.matmul(
                        ps01[qq][:, j, :],
                        winv_sb[:, qq, c, j * 128:(j + 1) * 128],
                        t1f_sb[:, c, :],
                        start=(c == 0 and j == 0),
                        stop=(c == NSLOT - 1),
                        skip_group_check=True,
                    )

            for gi in range(16):
                xq = tsp.tile([128, 4, H], BF16, tag="ts", name=f"xq{gi}")
                nc.scalar.dma_start(
                    out=xq[:],
                    in_=xT.ap()[gi * F:(gi + 1) * F, :].rearrange(
                        "(kc p) m -> p kc m", p=128
                    ),
                )
                for hf in range(2):
                    pt = psq.tile(
                        [128, 2, C], F32, tag="psA", name=f"pt1_{gi}_{hf}"
                    )
                    for j in range(2):
                        jj = 2 * hf + j
                        for kc in range(4):
                            nc.tensor.matmul(
                                pt[:, j, :],
                                xq[:, kc, jj * 128:(jj + 1) * 128],
                                w1_sb[:, kc, :],
                                start=(j == 0 and kc == 0),
                                stop=(kc == 3),
                                skip_group_check=True,
                            )
                        nc.vector.tensor_copy(
                            t1f_sb[:, 4 * gi + jj, :], pt[:, j, :]
                        )
                if gi >= 3:
                    for c in range(4 * (gi - 3), 4 * (gi - 2)):
                        s1q_mm(0, c)
                if gi >= 9:
                    for c in range(8 * (gi - 9), 8 * (gi - 8)):
                        s1q_mm(1, c)
            for c in range(4 * 13, NSLOT):
                s1q_mm(0, c)
            for c in range(8 * 7, NSLOT):
                s1q_mm(1, c)

            nc.scalar.dma_start(
                out=w2_sb[:], in_=w2.ap().rearrange("(kc p) n -> p kc n", p=128)
            )
            nc.scalar.dma_start(
                out=f1_sb[:], in_=f1.ap().rearrange("(mt p) -> p mt", p=128)
            )
            nc.scalar.dma_start(
                out=f2_sb[:], in_=f2.ap().rearrange("(mt p) -> p mt", p=128)
            )


            # ======= s1 = Winv @ t1 (all SBUF), col-quarter passes; AG halves =
            s_sb1 = stgp.tile([128, 8, C], BF16, tag="stg", name="s_sb1")
            for qq in range(2):
                for j in range(2):
                    nc.vector.tensor_scalar_mul(
                        s_sb1[:, 2 * qq + j, :],
                        ps01[qq][:, j, :],
                        f1_sb[:, 2 * qq + j:2 * qq + j + 1],
                    )
            h = 0
            nc.scalar.dma_start(
                out=s1h_d[h][:, :].rearrange("(p k) n -> p k n", p=128),
                in_=s_sb1[:, 0:4, :],
            )
            all_gather(s1h_d[h], s1g_d[h])
            for q in range(2, 4):
                ps = psq.tile([128, 2, C], F32, tag="psA", name=f"ps1_{q}")
                for p in range(NSLOT):
                    for j in range(2):
                        nc.tensor.matmul(
                            ps[:, j, :],
                            winv_sb[:, q, p, j * 128:(j + 1) * 128],
                            t1f_sb[:, p, :],
                            start=(p == 0 and j == 0),
                            stop=(p == NSLOT - 1),
                            skip_group_check=True,
                        )
                for j in range(2):
                    nc.vector.tensor_scalar_mul(
                        s_sb1[:, 2 * q + j, :],
                        ps[:, j, :],
                        f1_sb[:, 2 * q + j:2 * q + j + 1],
                    )
                if q == 3:
                    nc.scalar.dma_start(
                        out=s1h_d[1][:, :].rearrange("(p k) n -> p k n", p=128),
                        in_=s_sb1[:, 4:8, :],
                    )
                    all_gather(s1h_d[1], s1g_d[1])

            # ---- o phase: out_loc = (w[rows]*f) @ s_full, slot-pipelined ----
            # consumes gathered rank-halves (256KB contiguous) in sigma order.
            def o_phase(sg_d, drain_cb, name, head, side=None):
                po = [
                    psO.tile([128, R], F32, tag="po", name=f"po_{name}{ch}")
                    for ch in range(2)
                ]
                wt_tiles = {}

                def load_wt(g):
                    t = wtp.tile([128, 4, R], BF16, tag="wt", name=f"wt_{name}{g}")
                    nc.sync.dma_start(
                        out=t[:],
                        in_=wT.ap()[g * 512:(g + 1) * 512, :].rearrange(
                            "(p kc) m -> p kc m", p=128
                        ),
                    )
                    wt_tiles[g] = t

                def wt_of(c, mh):
                    # slots 0..15 come from the pre-staged 4MB head tile
                    if c < 16:
                        return head[:, c, mh * 512:(mh + 1) * 512]
                    g, jj = divmod(c, 4)
                    return wt_tiles[g][:, jj, mh * 512:(mh + 1) * 512]

                for h in range(2):
                    for rk in range(NCORES):
                        g = h * 8 + rk
                        sgt = tsp.tile(
                            [128, 4, C], BF16, tag="ts", name=f"so_{name}_{g}"
                        )
                        nc.scalar.dma_start(
                            out=sgt[:],
                            in_=sg_d[h][rk * H:(rk + 1) * H, :].rearrange(
                                "(p k) n -> p k n", p=128
                            ),
                        )
                        if 4 <= g + 2 < 16:
                            load_wt(g + 2)
                        if side is not None and g in (4, 7, 10, 13):
                            side((g - 4) // 3)
                        for jj in range(4):
                            c = 4 * g + jj
                            for ch in range(2):
                                for mh in range(2):
                                    nc.tensor.matmul(
                                        po[ch][:, mh * 512:(mh + 1) * 512],
                                        sgt[:, jj, ch * 128:(ch + 1) * 128],
                                        wt_of(c, mh),
                                        start=(c == 0),
                                        stop=(c == NSLOT - 1),
                                        skip_group_check=True,
                                    )
                for ch in range(2):
                    drain_cb(ch, po[ch])

            # ================= layer 1 out =================
            _h1 = {}

            def relu_drain(ch, po):
                if "t" not in _h1:
                    _h1["t"] = stgp.tile(
                        [128, C // 128, R], BF16, tag="stg", name="h1T_sb"
                    )
                h1T_sb = _h1["t"]
                for mh in range(2):
                    nc.vector.tensor_scalar_max(
                        h1T_sb[:, ch, mh * 512:(mh + 1) * 512],
                        po[:, mh * 512:(mh + 1) * 512],
                        0.0,
                    )

            wt_head1 = t1fp.tile([128, 16, R], BF16, tag="big", name="wt_head1")
            for _i in range(4):
                nc.sync.dma_start(
                    out=wt_head1[:, 4 * _i:4 * (_i + 1), :],
                    in_=wT.ap()[_i * 512:(_i + 1) * 512, :].rearrange(
                        "(p kc) m -> p kc m", p=128
                    ),
                )
            wt_head2 = t1fp.tile([128, 16, R], BF16, tag="big", name="wt_head2")

            def o1_side(i):
                nc.sync.dma_start(
                    out=wt_head2[:, 4 * i:4 * (i + 1), :],
                    in_=wT.ap()[i * 512:(i + 1) * 512, :].rearrange(
                        "(p kc) m -> p kc m", p=128
                    ),
                )

            o_phase(s1g_d, relu_drain, "o1", wt_head1, side=o1_side)
            h1T_sb = _h1["t"]

            # ======= t2 = relu(o1) @ W2 (local rows), AG per half =======
            t_sb2 = stgp.tile([128, 8, C], BF16, tag="stg", name="t_sb2")
            for h in range(2):
                for q2 in range(2):
                    q = 2 * h + q2
                    pt = psq.tile([128, 2, C], F32, tag="psA", name=f"pt2_{q}")
                    for j in range(2):
                        mt = 2 * q + j
                        for kc in range(2):
                            nc.tensor.matmul(
                                pt[:, j, :],
                                h1T_sb[:, kc, mt * 128:(mt + 1) * 128],
                                w2_sb[:, kc, :],
                                start=(j == 0 and kc == 0),
                                stop=(kc == 1),
                                skip_group_check=True,
                            )
                        nc.vector.tensor_copy(t_sb2[:, mt, :], pt[:, j, :])
                nc.scalar.dma_start(
                    out=t2h_d[h][:, :].rearrange("(p k) n -> p k n", p=128),
                    in_=t_sb2[:, 4 * h:4 * h + 4, :],
                )
                all_gather(t2h_d[h], t2g_d[h])

            # ======= s2 = Winv @ t2_full, two output-half passes =======
            # each pass sweeps all 64 slots for rows hp*512..; its half is
            # stored + AllGather'd while the other pass computes, so o2
            # never waits on E4.  Granule re-reads are cheap (winv resident).
            s_sb2 = stgp.tile([128, 8, C], BF16, tag="stg", name="s_sb2")
            for hp in range(2):
                psh = [
                    psq.tile([128, 2, C], F32, tag="psA", name=f"ps2_{hp}_{i}")
                    for i in range(2)
                ]
                for h in range(2):
                    for rk in range(NCORES):
                        g = h * 8 + rk
                        tsg = tsp.tile(
                            [128, 4, C], BF16, tag="ts", name=f"ts2_{hp}_{g}"
                        )
                        (nc.scalar if g % 2 else nc.sync).dma_start(
                            out=tsg[:],
                            in_=t2g_d[h][rk * H:(rk + 1) * H, :].rearrange(
                                "(p k) n -> p k n", p=128
                            ),
                        )
                        for jj in range(4):
                            c = 4 * g + jj
                            for m4 in range(4):
                                mt = 4 * hp + m4
                                nc.tensor.matmul(
                                    psh[m4 // 2][:, m4 % 2, :],
                                    winv_sb[
                                        :, mt // 2, c,
                                        (mt % 2) * 128:(mt % 2 + 1) * 128,
                                    ],
                                    tsg[:, jj, :],
                                    start=(c == 0 and m4 % 2 == 0),
                                    stop=(c == NSLOT - 1),
                                    skip_group_check=True,
                                )
                for m4 in range(4):
                    mt = 4 * hp + m4
                    nc.vector.tensor_scalar_mul(
                        s_sb2[:, mt, :],
                        psh[m4 // 2][:, m4 % 2, :],
                        f2_sb[:, mt:mt + 1],
                    )
                nc.scalar.dma_start(
                    out=s2h_d[hp][:, :].rearrange("(p k) n -> p k n", p=128),
                    in_=s_sb2[:, 4 * hp:4 * hp + 4, :],
                )
                all_gather(s2h_d[hp], s2g_d[hp])

            # ================= layer 2 out =================
            # out_st reuses a "wt" slot; allocated lazily AFTER o2's last wT
            # tile so the ring rotation never makes a wT load wait on the
            # final output stores.
            _oh = {}

            def out_drain(ch, po):
                if "t" not in _oh:
                    _oh["t"] = wtp.tile([128, 2, R], F32, tag="wt", name="out_st")
                out_st = _oh["t"]
                for mh in range(2):
                    nc.vector.tensor_copy(
                        out_st[:, ch, mh * 512:(mh + 1) * 512],
                        po[:, mh * 512:(mh + 1) * 512],
                    )
                    nc.scalar.dma_start(
                        out=outT.ap()[
                            ch * 128:(ch + 1) * 128, mh * 512:(mh + 1) * 512
                        ],
                        in_=out_st[:, ch, mh * 512:(mh + 1) * 512],
                    )

            o_phase(s2g_d, out_drain, "o2", wt_head2)

    nc.compile()
    return nc


_NC_CACHE = {}


def _get_nc():
    if "nc" not in _NC_CACHE:
        _NC_CACHE["nc"] = build_kernel()
    return _NC_CACHE["nc"]


# global sigma order: half-major, rank-major 512-row blocks
_PERM = np.concatenate(
    [
        np.arange(rk * R + h * H, rk * R + h * H + H)
        for h in range(2)
        for rk in range(NCORES)
    ]
)


def make_in_maps(input, wavelets, wavelets_inv, W1, W2, filter1, filter2):
    input = np.asarray(input, np.float32)
    wavelets = np.asarray(wavelets, np.float32)
    wavelets_inv = np.asarray(wavelets_inv, np.float32)
    W1b = np.ascontiguousarray(np.asarray(W1, np.float32)).astype(NP_BF16)
    W2b = np.ascontiguousarray(np.asarray(W2, np.float32)).astype(NP_BF16)
    filter1 = np.asarray(filter1, np.float32)
    filter2 = np.asarray(filter2, np.float32)

    xT_pi = input.T[:, _PERM].astype(NP_BF16)
    # granule-major [16*F, H]: granule gi contiguous; identical on every core
    xT_g = np.ascontiguousarray(
        np.concatenate([xT_pi[:, g * H:(g + 1) * H] for g in range(16)], axis=0)
    )
    in_maps = []
    for i in range(NCORES):
        r0, r1 = i * R, (i + 1) * R
        wvT_pi = wavelets_inv[r0:r1].T[_PERM].astype(NP_BF16)
        # col-quarter-major [4*N, Q], each 2MB piece partition-major so the
        # fill DMA reads 16KB/partition contiguous (512B units ran ~160GB/s)
        winvT_i = np.ascontiguousarray(
            np.concatenate(
                [wvT_pi[:, q * Q:(q + 1) * Q] for q in range(4)], axis=0
            )
            .reshape(8, 32, 128, Q)
            .transpose(0, 2, 1, 3)
            .reshape(4 * N, Q)
        )
        # wT: each 512-row slab partition-major (8KB/partition units)
        wT_i = np.ascontiguousarray(
            wavelets[r0:r1].T[_PERM]
            .astype(NP_BF16)
            .reshape(16, 4, 128, R)
            .transpose(0, 2, 1, 3)
            .reshape(N, R)
        )
        in_maps.append(
            {
                "xT": xT_g,
                "w1": W1b,
                "w2": W2b,
                "winvT": winvT_i,
                "wT": wT_i,
                "f1": np.ascontiguousarray(filter1[r0:r1]),
                "f2": np.ascontiguousarray(filter2[r0:r1]),
            }
        )
    return in_maps


def run(in_maps, trace=False, **kw):
    nc = _get_nc()
    return bass_utils.run_bass_kernel_spmd(
        nc, in_maps, core_ids=list(range(NCORES)), trace=trace, **kw
    )


def kernel(input, wavelets, wavelets_inv, W1, W2, filter1, filter2):
    in_maps = make_in_maps(
        input, wavelets, wavelets_inv, W1, W2, filter1, filter2
    )
    res = run(in_maps)
    out = np.empty((N, C), np.float32)
    for i in range(NCORES):
        out[i * R:(i + 1) * R, :] = res.results[i]["outT"].T
    return out


# revision 29
# speedup vs baseline: 1.0327x; 1.0261x over previous
"""Graph Wavelet NN (2-layer) Trainium2 kernel, 8-core row-parallel.

Math per layer: out = (wavelets * f) @ (wavelets_inv @ (x @ W)); the filter is
folded into a row-scale of the small spectral tensor s computed per core.

Final design (measured ~444-460us vs the 477us v1 baseline on the same
trace-based metric; run-to-run variance tracks the ~40-55us cross-core launch
skew absorbed by the first collective):
- t1 = x @ W1 computed FULLY REPLICATED per core, with s1's first TWO
  column-quarter passes interleaved into the t1 granule loop (lagged 3/9
  granules behind the xq stream).  The PE never idles long enough to
  re-throttle the HAM clock gate, layer 1 needs no input exchange, and
  E2-h0 (the skew-absorbing first AllGather) fires as early as possible.
- winvT is SBUF-RESIDENT (16MB, filled during t1 from a host-prepared
  col-quarter-major layout in 8 contiguous 2MB DMAs, reused by s2); wT is
  streamed during o1 and o2, with the first 4MB of each o-phase pre-staged
  into t1f's dead SBUF slot (wt_head1/2, head2 trickled into o1's stream).
- ONE total order sigma = (half, rank) over the 64 contraction blocks is
  baked into the host layouts of xT, winvT, wT; every DMA is a contiguous
  block read and every consumer walks gathered rank-halves (256KB reads,
  own rank included) in AllGather arrival order -> fully static program,
  no dynamic-offset DMAs (their register setup paced earlier versions).
- All exchanges are HALVES (256KB in / 2MB out per AllGather): s1 AGs each
  half as its passes finish; t2 AGs per half; s2 runs as TWO output-half
  passes so E4-h0 flies mid-s2 and o2 starts stall-free.  o1/o2 are
  slot-pipelined single passes.  A dummy AllGather at t=0 absorbs the
  one-time ncfw stream init.
- Collectives alone on gpsimd; winv/wT streams on sync; xq/gather reads and
  stores on scalar (s2 granule reads alternate scalar/sync).  bf16 matmuls,
  fp32 PSUM; PSUM: 4x2KB "psA" accumulators + 2x4KB o-phase tiles = 16KB.

Verified on HW: rel err 6.345e-3.
"""

import sys

if "/opt/trn_rl_repo" not in sys.path:
    sys.path.insert(0, "/opt/trn_rl_repo")

import numpy as np
import ml_dtypes

import concourse.bass as bass
import concourse.mybir as mybir
import concourse.tile as tile
from concourse import bacc, bass_utils

N = 8192
F = 512
C = 256
NCORES = 8
R = N // NCORES          # 1024 rows per core
H = R // 2               # 512-row half (exchange granule)
Q = R // 4               # 256-col quarter (s-phase output passes)
NSLOT = N // 128         # 64 contraction slots of 128 rows

F32 = mybir.dt.float32
BF16 = mybir.dt.bfloat16
NP_BF16 = ml_dtypes.bfloat16


def build_kernel(sim_single_core=False):
    nc = bacc.Bacc(
        "TRN2",
        target_bir_lowering=False,
        debug=False,
        num_devices=1 if sim_single_core else NCORES,
    )

    xT = nc.dram_tensor("xT", [16 * F, H], BF16, kind="ExternalInput")
    w1 = nc.dram_tensor("w1", [F, C], BF16, kind="ExternalInput")
    w2 = nc.dram_tensor("w2", [C, C], BF16, kind="ExternalInput")
    winvT = nc.dram_tensor("winvT", [4 * N, Q], BF16, kind="ExternalInput")
    wT = nc.dram_tensor("wT", [N, R], BF16, kind="ExternalInput")
    f1 = nc.dram_tensor("f1", [R], F32, kind="ExternalInput")
    f2 = nc.dram_tensor("f2", [R], F32, kind="ExternalInput")
    outT = nc.dram_tensor("outT", [C, R], F32, kind="ExternalOutput")

    rg = [list(range(NCORES))]

    with tile.TileContext(nc) as tc:
        with (
            tc.tile_pool(name="dram", bufs=1, space="DRAM") as dram,
            tc.tile_pool(name="const", bufs=1) as const,
            tc.tile_pool(name="stgp", bufs=2) as stgp,
            tc.tile_pool(name="t1fp", bufs=1) as t1fp,
            tc.tile_pool(name="wtp", bufs=3) as wtp,
            tc.tile_pool(name="tsp", bufs=3) as tsp,
            tc.tile_pool(name="psq", bufs=4, space="PSUM") as psq,
            tc.tile_pool(name="psO", bufs=2, space="PSUM") as psO,
        ):
            # ---- DRAM exchange buffers (halves) ----
            def mk_pair(nm):
                ins, outs = [], []
                for b in range(2):
                    ins.append(dram.tile([H, C], BF16, name=f"{nm}{b}_d"))
                    outs.append(
                        dram.tile(
                            [NCORES * H, C], BF16,
                            addr_space="Local" if sim_single_core else "Shared",
                            name=f"{nm}{b}g_d",
                        )
                    )
                return ins, outs

            s1h_d, s1g_d = mk_pair("s1")
            t2h_d, t2g_d = mk_pair("t2")
            s2h_d, s2g_d = mk_pair("s2")

            # dummy collective: starts the ncfw stream init at t~0 so the
            # first real AllGather doesn't eat the one-time cost.
            dum_i = dram.tile([Q, 1], BF16, name="dum_i")
            dum_o = dram.tile(
                [NCORES * Q, 1], BF16,
                addr_space="Local" if sim_single_core else "Shared",
                name="dum_o",
            )

            # ---- persistent SBUF ----
            winv_sb = const.tile([128, 4, NSLOT, Q], BF16)  # 128KB/part
            t1f_sb = t1fp.tile(
                [128, NSLOT, C], BF16, tag="big", name="t1f_sb"
            )  # full t1, sigma order
            w1_sb = const.tile([128, F // 128, C], BF16)
            w2_sb = const.tile([128, C // 128, C], BF16)
            f1_sb = const.tile([128, 8], F32)
            f2_sb = const.tile([128, 8], F32)

            def all_gather(in_d, out_d):
                if sim_single_core:
                    rows = in_d.shape[0]
                    for rr in range(NCORES):
                        nc.sync.dma_start(
                            out=out_d[rr * rows:(rr + 1) * rows, :], in_=in_d[:, :]
                        )
                else:
                    nc.gpsimd.collective_compute(
                        "AllGather",
                        mybir.AluOpType.bypass,
                        replica_groups=rg,
                        ins=[in_d.opt()],
                        outs=[out_d.opt()],
                    )

            all_gather(dum_i, dum_o)

            nc.scalar.dma_start(
                out=w1_sb[:], in_=w1.ap().rearrange("(kc p) n -> p kc n", p=128)
            )

            # winv fill: col-quarter-major 2MB pieces, each contiguous.
            for q in range(4):
                for g2 in range(2):
                    nc.sync.dma_start(
                        out=winv_sb[:, q, g2 * 32:(g2 + 1) * 32, :],
                        in_=winvT.ap()[
                            q * N + g2 * 4096:q * N + (g2 + 1) * 4096, :
                        ].rearrange("(p kc) m -> p kc m", p=128),
                    )

            # ======= t1 = x @ W1, fully replicated, staged in sigma order =====
            # xq granule gi = 512 sigma-columns (4 slots), contiguous 512KB.
            # s1's pass-0 matmuls are interleaved 3 granules behind t1 so the
            # PE never idles between xq arrivals (idle gaps re-throttle the
            # HAM clock gate and were running t1 at half rate).
            ps01 = [
                psq.tile([128, 2, C], F32, tag="psA", name=f"ps1_{qq}")
                for qq in range(2)
            ]

            def s1q_mm(qq, c):
                for j in range(2):
                    nc.tensor.matmul(
                        ps01[qq][:, j, :],
                        winv_sb[:, qq, c, j * 128:(j + 1) * 128],
                        t1f_sb[:, c, :],
                        start=(c == 0 and j == 0),
                        stop=(c == NSLOT - 1),
                        skip_group_check=True,
                    )

            for gi in range(16):
                xq = tsp.tile([128, 4, H], BF16, tag="ts", name=f"xq{gi}")
                nc.scalar.dma_start(
                    out=xq[:],
                    in_=xT.ap()[gi * F:(gi + 1) * F, :].rearrange(
                        "(kc p) m -> p kc m", p=128
                    ),
                )
                for hf in range(2):
                    pt = psq.tile(
                        [128, 2, C], F32, tag="psA", name=f"pt1_{gi}_{hf}"
                    )
                    for j in range(2):
                        jj = 2 * hf + j
                        for kc in range(4):
                            nc.tensor.matmul(
                                pt[:, j, :],
                                xq[:, kc, jj * 128:(jj + 1) * 128],
                                w1_sb[:, kc, :],
                                start=(j == 0 and kc == 0),
                                stop=(kc == 3),
                                skip_group_check=True,
                            )
                        nc.vector.tensor_copy(
                            t1f_sb[:, 4 * gi + jj, :], pt[:, j, :]
                        )
                if gi >= 3:
                    for c in range(4 * (gi - 3), 4 * (gi - 2)):
                        s1q_mm(0, c)
                if gi >= 9:
                    for c in range(8 * (gi - 9), 8 * (gi - 8)):
                        s1q_mm(1, c)
            for c in range(4 * 13, NSLOT):
                s1q_mm(0, c)
            for c in range(8 * 7, NSLOT):
                s1q_mm(1, c)

            nc.scalar.dma_start(
                out=w2_sb[:], in_=w2.ap().rearrange("(kc p) n -> p kc n", p=128)
            )
            nc.scalar.dma_start(
                out=f1_sb[:], in_=f1.ap().rearrange("(mt p) -> p mt", p=128)
            )
            nc.scalar.dma_start(
                out=f2_sb[:], in_=f2.ap().rearrange("(mt p) -> p mt", p=128)
            )


            # ======= s1 = Winv @ t1 (all SBUF), col-quarter passes; AG halves =
            s_sb1 = stgp.tile([128, 8, C], BF16, tag="stg", name="s_sb1")
            for qq in range(2):
                for j in range(2):
                    nc.vector.tensor_scalar_mul(
                        s_sb1[:, 2 * qq + j, :],
                        ps01[qq][:, j, :],
                        f1_sb[:, 2 * qq + j:2 * qq + j + 1],
                    )
            h = 0
            nc.scalar.dma_start(
                out=s1h_d[h][:, :].rearrange("(p k) n -> p k n", p=128),
                in_=s_sb1[:, 0:4, :],
            )
            all_gather(s1h_d[h], s1g_d[h])
            for q in range(2, 4):
                ps = psq.tile([128, 2, C], F32, tag="psA", name=f"ps1_{q}")
                for p in range(NSLOT):
                    for j in range(2):
                        nc.tensor.matmul(
                            ps[:, j, :],
                            winv_sb[:, q, p, j * 128:(j + 1) * 128],
                            t1f_sb[:, p, :],
                            start=(p == 0 and j == 0),
                            stop=(p == NSLOT - 1),
                            skip_group_check=True,
                        )
                for j in range(2):
                    nc.vector.tensor_scalar_mul(
                        s_sb1[:, 2 * q + j, :],
                        ps[:, j, :],
                        f1_sb[:, 2 * q + j:2 * q + j + 1],
                    )
                if q == 3:
                    nc.scalar.dma_start(
                        out=s1h_d[1][:, :].rearrange("(p k) n -> p k n", p=128),
                        in_=s_sb1[:, 4:8, :],
                    )
                    all_gather(s1h_d[1], s1g_d[1])

            # ---- o phase: out_loc = (w[rows]*f) @ s_full, slot-pipelined ----
            # consumes gathered rank-halves (256KB contiguous) in sigma order.
            def o_phase(sg_d, drain_cb, name, head, side=None):
                po = [
                    psO.tile([128, R], F32, tag="po", name=f"po_{name}{ch}")
                    for ch in range(2)
                ]
                wt_tiles = {}

                def load_wt(g):
                    t = wtp.tile([128, 4, R], BF16, tag="wt", name=f"wt_{name}{g}")
                    nc.sync.dma_start(
                        out=t[:],
                        in_=wT.ap()[g * 512:(g + 1) * 512, :].rearrange(
                            "(p kc) m -> p kc m", p=128
                        ),
                    )
                    wt_tiles[g] = t

                def wt_of(c, mh):
                    # slots 0..15 come from the pre-staged 4MB head tile
                    if c < 16:
                        return head[:, c, mh * 512:(mh + 1) * 512]
                    g, jj = divmod(c, 4)
                    return wt_tiles[g][:, jj, mh * 512:(mh + 1) * 512]

                for h in range(2):
                    for rk in range(NCORES):
                        g = h * 8 + rk
                        sgt = tsp.tile(
                            [128, 4, C], BF16, tag="ts", name=f"so_{name}_{g}"
                        )
                        nc.scalar.dma_start(
                            out=sgt[:],
                            in_=sg_d[h][rk * H:(rk + 1) * H, :].rearrange(
                                "(p k) n -> p k n", p=128
                            ),
                        )
                        if 4 <= g + 2 < 16:
                            load_wt(g + 2)
                        if side is not None and g in (4, 7, 10, 13):
                            side((g - 4) // 3)
                        for jj in range(4):
                            c = 4 * g + jj
                            for ch in range(2):
                                for mh in range(2):
                                    nc.tensor.matmul(
                                        po[ch][:, mh * 512:(mh + 1) * 512],
                                        sgt[:, jj, ch * 128:(ch + 1) * 128],
                                        wt_of(c, mh),
                                        start=(c == 0),
                                        stop=(c == NSLOT - 1),
                                        skip_group_check=True,
                                    )
                for ch in range(2):
                    drain_cb(ch, po[ch])

            # ================= layer 1 out =================
            _h1 = {}

            def relu_drain(ch, po):
                if "t" not in _h1:
                    _h1["t"] = stgp.tile(
                        [128, C // 128, R], BF16, tag="stg", name="h1T_sb"
                    )
                h1T_sb = _h1["t"]
                for mh in range(2):
                    nc.vector.tensor_scalar_max(
                        h1T_sb[:, ch, mh * 512:(mh + 1) * 512],
                        po[:, mh * 512:(mh + 1) * 512],
                        0.0,
                    )

            wt_head1 = t1fp.tile([128, 16, R], BF16, tag="big", name="wt_head1")
            for _i in range(4):
                nc.sync.dma_start(
                    out=wt_head1[:, 4 * _i:4 * (_i + 1), :],
                    in_=wT.ap()[_i * 512:(_i + 1) * 512, :].rearrange(
                        "(p kc) m -> p kc m", p=128
                    ),
                )
            wt_head2 = t1fp.tile([128, 16, R], BF16, tag="big", name="wt_head2")

            def o1_side(i):
                nc.sync.dma_start(
                    out=wt_head2[:, 4 * i:4 * (i + 1), :],
                    in_=wT.ap()[i * 512:(i + 1) * 512, :].rearrange(
                        "(p kc) m -> p kc m", p=128
                    ),
                )

            o_phase(s1g_d, relu_drain, "o1", wt_head1, side=o1_side)
            h1T_sb = _h1["t"]

            # ======= t2 = relu(o1) @ W2 (local rows), AG per half =======
            t_sb2 = stgp.tile([128, 8, C], BF16, tag="stg", name="t_sb2")
            for h in range(2):
                for q2 in range(2):
                    q = 2 * h + q2
                    pt = psq.tile([128, 2, C], F32, tag="psA", name=f"pt2_{q}")
                    for j in range(2):
                        mt = 2 * q + j
                        for kc in range(2):
                            nc.tensor.matmul(
                                pt[:, j, :],
                                h1T_sb[:, kc, mt * 128:(mt + 1) * 128],
                                w2_sb[:, kc, :],
                                start=(j == 0 and kc == 0),
                                stop=(kc == 1),
                                skip_group_check=True,
                            )
                        nc.vector.tensor_copy(t_sb2[:, mt, :], pt[:, j, :])
                nc.scalar.dma_start(
                    out=t2h_d[h][:, :].rearrange("(p k) n -> p k n", p=128),
                    in_=t_sb2[:, 4 * h:4 * h + 4, :],
                )
                all_gather(t2h_d[h], t2g_d[h])

            # ======= s2 = Winv @ t2_full, two output-half passes =======
            # each pass sweeps all 64 slots for rows hp*512..; its half is
            # stored + AllGather'd while the other pass computes, so o2
            # never waits on E4.  Granule re-reads are cheap (winv resident).
            s_sb2 = stgp.tile([128, 8, C], BF16, tag="stg", name="s_sb2")
            for hp in range(2):
                psh = [
                    psq.tile([128, 2, C], F32, tag="psA", name=f"ps2_{hp}_{i}")
                    for i in range(2)
                ]
                for h in range(2):
                    for rk in range(NCORES):
                        g = h * 8 + rk
                        tsg = tsp.tile(
                            [128, 4, C], BF16, tag="ts", name=f"ts2_{hp}_{g}"
                        )
                        (nc.scalar if g % 2 else nc.sync).dma_start(
                            out=tsg[:],
                            in_=t2g_d[h][rk * H:(rk + 1) * H, :].rearrange(
                                "(p k) n -> p k n", p=128
                            ),
                        )
                        for jj in range(4):
                            c = 4 * g + jj
                            for m4 in range(4):
                                mt = 4 * hp + m4
                                nc.tensor.matmul(
                                    psh[m4 // 2][:, m4 % 2, :],
                                    winv_sb[
                                        :, mt // 2, c,
                                        (mt % 2) * 128:(mt % 2 + 1) * 128,
                                    ],
                                    tsg[:, jj, :],
                                    start=(c == 0 and m4 % 2 == 0),
                                    stop=(c == NSLOT - 1),
                                    skip_group_check=True,
                                )
                for m4 in range(4):
                    mt = 4 * hp + m4
                    nc.vector.tensor_scalar_mul(
                        s_sb2[:, mt, :],
                        psh[m4 // 2][:, m4 % 2, :],
                        f2_sb[:, mt:mt + 1],
                    )
                nc.scalar.dma_start(
                    out=s2h_d[hp][:, :].rearrange("(p k) n -> p k n", p=128),
                    in_=s_sb2[:, 4 * hp:4 * hp + 4, :],
                )
                all_gather(s2h_d[hp], s2g_d[hp])

            # ================= layer 2 out =================
            # out_st reuses a "wt" slot; allocated lazily AFTER o2's last wT
            # tile so the ring rotation never makes a wT load wait on the
            # final output stores.
            _oh = {}

            def out_drain(ch, po):
                if "t" not in _oh:
                    _oh["t"] = wtp.tile([128, 2, R], F32, tag="wt", name="out_st")
                out_st = _oh["t"]
                for mh in range(2):
                    nc.vector.tensor_copy(
                        out_st[:, ch, mh * 512:(mh + 1) * 512],
                        po[:, mh * 512:(mh + 1) * 512],
                    )
                    nc.scalar.dma_start(
                        out=outT.ap()[
                            ch * 128:(ch + 1) * 128, mh * 512:(mh + 1) * 512
                        ],
                        in_=out_st[:, ch, mh * 512:(mh + 1) * 512],
                    )

            o_phase(s2g_d, out_drain, "o2", wt_head2)

    nc.compile()
    return nc


_NC_CACHE = {}


def _get_nc():
    if "nc" not in _NC_CACHE:
        _NC_CACHE["nc"] = build_kernel()
    return _NC_CACHE["nc"]


# global sigma order: half-major, rank-major 512-row blocks
_PERM = np.concatenate(
    [
        np.arange(rk * R + h * H, rk * R + h * H + H)
        for h in range(2)
        for rk in range(NCORES)
    ]
)


def make_in_maps(input, wavelets, wavelets_inv, W1, W2, filter1, filter2):
    input = np.asarray(input, np.float32)
    wavelets = np.asarray(wavelets, np.float32)
    wavelets_inv = np.asarray(wavelets_inv, np.float32)
    W1b = np.ascontiguousarray(np.asarray(W1, np.float32)).astype(NP_BF16)
    W2b = np.ascontiguousarray(np.asarray(W2, np.float32)).astype(NP_BF16)
    filter1 = np.asarray(filter1, np.float32)
    filter2 = np.asarray(filter2, np.float32)

    xT_pi = input.T[:, _PERM].astype(NP_BF16)
    # granule-major [16*F, H]: granule gi contiguous; identical on every core
    xT_g = np.ascontiguousarray(
        np.concatenate([xT_pi[:, g * H:(g + 1) * H] for g in range(16)], axis=0)
    )
    in_maps = []
    for i in range(NCORES):
        r0, r1 = i * R, (i + 1) * R
        wvT_pi = wavelets_inv[r0:r1].T[_PERM].astype(NP_BF16)
        # col-quarter-major [4*N, Q], each 2MB piece partition-major so the
        # fill DMA reads 16KB/partition contiguous (512B units ran ~160GB/s)
        winvT_i = np.ascontiguousarray(
            np.concatenate(
                [wvT_pi[:, q * Q:(q + 1) * Q] for q in range(4)], axis=0
            )
            .reshape(8, 32, 128, Q)
            .transpose(0, 2, 1, 3)
            .reshape(4 * N, Q)
        )
        # wT: each 512-row slab partition-major (8KB/partition units)
        wT_i = np.ascontiguousarray(
            wavelets[r0:r1].T[_PERM]
            .astype(NP_BF16)
            .reshape(16, 4, 128, R)
            .transpose(0, 2, 1, 3)
            .reshape(N, R)
        )
        in_maps.append(
            {
                "xT": xT_g,
                "w1": W1b,
                "w2": W2b,
                "winvT": winvT_i,
                "wT": wT_i,
                "f1": np.ascontiguousarray(filter1[r0:r1]),
                "f2": np.ascontiguousarray(filter2[r0:r1]),
            }
        )
    return in_maps


def run(in_maps, trace=False, **kw):
    nc = _get_nc()
    return bass_utils.run_bass_kernel_spmd(
        nc, in_maps, core_ids=list(range(NCORES)), trace=trace, **kw
    )


def kernel(input, wavelets, wavelets_inv, W1, W2, filter1, filter2):
    in_maps = make_in_maps(
        input, wavelets, wavelets_inv, W1, W2, filter1, filter2
    )
    res = run(in_maps)
    out = np.empty((N, C), np.float32)
    for i in range(NCORES):
        out[i * R:(i + 1) * R, :] = res.results[i]["outT"].T
    return out
